# revision 2
# baseline (speedup 1.0000x reference)
"""Trainium2 Bass kernel for nn_MEPG_Loss (MEPG policy-gradient loss).

Math (forward only; stop_gradient is identity):
    h   = tanh(states[s,:,t] @ W1 + b1)                  [S,T,H]
    mu  = h @ W2 + b2                                    [S,T,A]
    ll[s,t] = -0.5*(||a[s,:,t]-mu||^2/SD + A*log(2*pi*SD))
    base = rewards.T - ALPHA*ll.T ; cum = base with row T-2 += row T-1
    A_hat = cum - log(0.5)
    out = einsum('ts,us->', A_hat, ll.T)/S
        = sum_s (sum_t A_hat[t,s]) * (sum_t ll[t,s]) / S

Only per-simulation reductions are needed:
    q_sum[s]  = sum_{t,d} (mu - a)^2,   q_last[s] = sum_d (mu - a)^2 at t=T-1
    R[s] = sum_t rewards,               r_last[s] = rewards[s,T-1]
(R/r_last come straight from host numpy; rewards are never sent to the device.)

Device pipeline, per core (256 sims as 64 quads of 4 sims):
    - states prepacked on host to [64, NQ*T] bf16; per 4-quad block one
      contiguous DMA per sim-slot j lands at SBUF partitions {32j..32j+16}
    - mm1: 4 row-tiled K=16 matmuls (concurrent via tile_position) fill a
      4-bank PSUM tile hp [128, 4*T] with h_pre for the whole quad
    - ScalarE: ONE tanh activation over all 2048 columns (bias=b1) -> h bf16
    - mm2: 4 col-tiled matmuls (lhsT=W2, start) write mu into hp[:, 0:T],
      reusing the first bank of the already-consumed h_pre tile
    - diff: 4 diag-tiled identity matmuls accumulate (b2 - a) onto mu
    - DVE: bn_stats + bn_aggr give per-partition mean/var of diff over T
      (=> sum of squares), plus a 1-col copy of diff at t=T-1
Final combine (tiny) is done on host in float64.

ScalarE is the bottleneck engine (tanh, 1 elem/lane/cycle): ~118 us floor.
Everything else is sized to stay below that and overlap fully.
"""

import os
import sys

import numpy as np

if not any(os.path.isdir(os.path.join(p, "concourse")) for p in sys.path if p):
    sys.path.insert(0, "/opt/trn_rl_repo")

import ml_dtypes

import concourse.bacc as bacc
import concourse.tile as tile
from concourse import mybir
from concourse.bass_utils import run_bass_kernel_spmd

# Problem constants (hardcoded per contract)
S, D, A, T, HID = 2048, 16, 4, 512, 128
N_CORES = 8
SS = S // N_CORES          # 256 sims per core
NQ = SS // 4               # 64 quads per core
QB = 4                     # quads per DMA block
NB = NQ // QB              # 16 blocks
SD_VAR = 0.04
ALPHA = 0.1
MAX_POSITION = 1.0

F32 = mybir.dt.float32
BF16 = mybir.dt.bfloat16
NP_BF16 = ml_dtypes.bfloat16


def _build_program():
    nc = bacc.Bacc("TRN2", target_bir_lowering=False, debug=False)

    stp_d = nc.dram_tensor("st_pre", [64, NQ * T], BF16, kind="ExternalInput").ap()
    atp_d = nc.dram_tensor("at_pre", [16, NQ * T], BF16, kind="ExternalInput").ap()
    w1f_d = nc.dram_tensor("w1full", [128, HID], BF16, kind="ExternalInput").ap()
    w2_d = nc.dram_tensor("w2", [HID, A], BF16, kind="ExternalInput").ap()
    id4_d = nc.dram_tensor("id4", [128, A], BF16, kind="ExternalInput").ap()
    b1_d = nc.dram_tensor("b1col", [HID, 1], F32, kind="ExternalInput").ap()

    mv_d = nc.dram_tensor("mv", [128, 2 * NQ], F32, kind="ExternalOutput").ap()
    ql_d = nc.dram_tensor("ql", [128, NQ], F32, kind="ExternalOutput").ap()

    with tile.TileContext(nc) as tc:
        with (
            tc.tile_pool(name="consts", bufs=1) as consts,
            tc.tile_pool(name="stp", bufs=3) as stp,
            tc.tile_pool(name="atp", bufs=3) as atp,
            tc.tile_pool(name="hsb", bufs=2) as hsb,
            tc.tile_pool(name="bstp", bufs=2) as bstp,
            tc.tile_pool(name="outs", bufs=1) as outp,
            tc.tile_pool(name="hpp", bufs=1, space="PSUM") as hpp,
        ):
            # dummy activation: forces the tanh table load at t~0, off the
            # critical path (no data dependence)
            dums = consts.tile([128, 1], F32, tag="dums")
            dumo = consts.tile([128, 1], F32, tag="dumo")
            nc.vector.memset(dums[:], 0.0)
            nc.scalar.activation(
                out=dumo[:], in_=dums[:],
                func=mybir.ActivationFunctionType.Tanh, scale=1.0,
            )

            # constants
            w1t = consts.tile([128, HID], BF16, tag="w1t")
            w2t = consts.tile([HID, A], BF16, tag="w2t")
            id4t = consts.tile([128, A], BF16, tag="id4t")
            b1t = consts.tile([HID, 1], F32, tag="b1t")
            nc.sync.dma_start(out=w1t[:], in_=w1f_d)
            nc.sync.dma_start(out=w2t[:], in_=w2_d)
            nc.sync.dma_start(out=id4t[:], in_=id4_d)
            nc.sync.dma_start(out=b1t[:], in_=b1_d)

            mv_sb = outp.tile([128, 2 * NQ], F32, tag="mv")
            ql_sb = outp.tile([128, NQ], F32, tag="ql")

            def _tail_quad(g, h, hp, at):
                q = g % QB
                mu = hp[:, 0:T]
                for j in range(4):
                    nc.tensor.matmul(
                        out=mu[32 * j:32 * j + A, :],
                        lhsT=w2t[:],
                        rhs=h[:, T * j:T * (j + 1)],
                        start=True, stop=False,
                        tile_position=(0, 32 * j),
                        skip_group_check=True,
                    )
                for j in range(4):
                    nc.tensor.matmul(
                        out=mu[32 * j:32 * j + A, :],
                        lhsT=id4t[32 * j:32 * j + A, :],
                        rhs=at[32 * j:32 * j + A, T * q:T * (q + 1)],
                        start=False, stop=True,
                        tile_position=(32 * j, 32 * j),
                        skip_group_check=True,
                    )
                sts = bstp.tile([128, 6], F32, tag="bst", name=f"bst_{g}")
                nc.vector.bn_stats(out=sts[:], in_=mu[:])
                nc.vector.bn_aggr(out=mv_sb[:, 2 * g:2 * g + 2], in_=sts[:])
                nc.vector.tensor_copy(ql_sb[:, g:g + 1], mu[:, T - 1:T])

            pipe = None
            for b in range(NB):
                c0 = QB * T * b
                st = stp.tile([128, QB * T], BF16, tag="st", name=f"st_{b}")
                at = atp.tile([128, QB * T], BF16, tag="at", name=f"at_{b}")
                for j in range(4):
                    nc.sync.dma_start(
                        out=st[32 * j:32 * j + D, :],
                        in_=stp_d[D * j:D * (j + 1), c0:c0 + QB * T],
                    )
                    nc.gpsimd.dma_start(
                        out=at[32 * j:32 * j + A, :],
                        in_=atp_d[A * j:A * (j + 1), c0:c0 + QB * T],
                    )
                for q in range(QB):
                    g = QB * b + q
                    hp = hpp.tile([128, 4 * T], F32, tag=f"hp{g % 2}",
                                  name=f"hp_{g}")
                    for j in range(4):
                        nc.tensor.matmul(
                            out=hp[:, T * j:T * (j + 1)],
                            lhsT=w1t[32 * j:32 * j + D, :],
                            rhs=st[32 * j:32 * j + D, T * q:T * (q + 1)],
                            start=True, stop=True,
                            tile_position=(32 * j, 0),
                        )
                    h = hsb.tile([128, 4 * T], BF16, tag="h", name=f"h_{g}")
                    nc.scalar.activation(
                        out=h[:], in_=hp[:],
                        func=mybir.ActivationFunctionType.Tanh,
                        bias=b1t[:], scale=1.0,
                    )
                    # software pipeline: tail work for the PREVIOUS quad so
                    # the PE finishes mm1(g+1) before ACT(g) completes
                    if pipe is not None:
                        _tail_quad(*pipe)
                    pipe = (g, h, hp, at)

            if pipe is not None:
                _tail_quad(*pipe)

            nc.sync.dma_start(out=mv_d, in_=mv_sb[:])
            nc.sync.dma_start(out=ql_d, in_=ql_sb[:])

    nc.finalize()
    return nc


_NC_CACHE = {}


def _get_program():
    if "nc" not in _NC_CACHE:
        _NC_CACHE["nc"] = _build_program()
    return _NC_CACHE["nc"]


def _make_consts(W1, b1, W2):
    w1full = np.zeros((128, HID), dtype=NP_BF16)
    id4 = np.zeros((128, A), dtype=NP_BF16)
    for j in range(4):
        w1full[32 * j:32 * j + D, :] = W1.astype(NP_BF16)
        for d in range(A):
            id4[32 * j + d, d] = 1.0
    return {
        "w1full": w1full,
        "w2": np.ascontiguousarray(W2.astype(NP_BF16)),
        "id4": id4,
        "b1col": np.ascontiguousarray(b1.astype(np.float32).reshape(HID, 1)),
    }


def kernel(states, actions, rewards, W1, b1, W2, b2, _run_kwargs=None):
    states = np.asarray(states, dtype=np.float32)
    actions = np.asarray(actions, dtype=np.float32)
    rewards = np.asarray(rewards, dtype=np.float32)
    W1 = np.asarray(W1, dtype=np.float32)
    b1 = np.asarray(b1, dtype=np.float32)
    W2 = np.asarray(W2, dtype=np.float32)
    b2 = np.asarray(b2, dtype=np.float32)

    consts = _make_consts(W1, b1, W2)

    # prepack per-core device layouts:
    #   st_pre[16j+dd, g*T+t] = states[core*SS + 4g+j, dd, t]   (bf16)
    #   at_pre[4j+d,  g*T+t] = b2[d] - actions[core*SS + 4g+j, d, t]  (bf16)
    st_all = states.reshape(N_CORES, SS // 4, 4, D, T)
    st_all = np.ascontiguousarray(st_all.transpose(0, 2, 3, 1, 4)).astype(NP_BF16)
    st_all = st_all.reshape(N_CORES, 64, NQ * T)
    aadj = b2[None, :, None] - actions
    at_all = aadj.reshape(N_CORES, SS // 4, 4, A, T)
    at_all = np.ascontiguousarray(at_all.transpose(0, 2, 3, 1, 4)).astype(NP_BF16)
    at_all = at_all.reshape(N_CORES, 16, NQ * T)

    in_maps = []
    for c in range(N_CORES):
        m = {"st_pre": st_all[c], "at_pre": at_all[c]}
        m.update(consts)
        in_maps.append(m)

    nc = _get_program()
    res = run_bass_kernel_spmd(nc, in_maps, core_ids=list(range(N_CORES)),
                               **(_run_kwargs or {}))
    results = res.results

    # host combine in float64
    C0 = -0.5 * A * np.log(2.0 * np.pi * SD_VAR)
    mx_pos = np.log(1.0 / (2.0 * MAX_POSITION))
    rew = rewards.astype(np.float64)
    R_all = rew.sum(axis=1)            # [S]
    rlast_all = rew[:, -1]             # [S]
    total = 0.0
    for c in range(N_CORES):
        mv = results[c]["mv"].astype(np.float64)      # [128, 2*NQ]
        qlv = results[c]["ql"].astype(np.float64)     # [128, NQ]
        mean = mv[:, 0::2]                            # [128, NQ]
        var = mv[:, 1::2]
        sumsq = T * (var + mean * mean)               # Sum_t diff^2 per (p, g)
        # partition p = 32j + d (d < A), sim s_local = 4g + j
        sel = sumsq.reshape(4, 32, NQ)[:, :A, :]      # [j, d, g]
        q_sum = sel.sum(axis=1).T.reshape(SS)         # s_local = 4g + j
        qsel = qlv.reshape(4, 32, NQ)[:, :A, :]
        q_last = (qsel ** 2).sum(axis=1).T.reshape(SS)
        sl = slice(SS * c, SS * (c + 1))
        L = -0.5 * q_sum / SD_VAR + T * C0
        ll_last = -0.5 * q_last / SD_VAR + C0
        A_sum = (R_all[sl] + rlast_all[sl]
                 - ALPHA * (L + ll_last) - T * mx_pos)
        total += np.sum(A_sum * L)
    out = np.float32(total / S)
    if _run_kwargs:
        _NC_CACHE["last_result"] = res
    return out


if __name__ == "__main__":
    rng = np.random.default_rng(0)
    inputs = {
        "states": rng.standard_normal((S, D, T), dtype=np.float32),
        "actions": rng.standard_normal((S, A, T), dtype=np.float32),
        "rewards": rng.standard_normal((S, T), dtype=np.float32),
        "W1": (rng.standard_normal((D, HID)) / np.sqrt(D)).astype(np.float32),
        "b1": np.zeros(HID, np.float32),
        "W2": (rng.standard_normal((HID, A)) / np.sqrt(HID)).astype(np.float32),
        "b2": np.zeros(A, np.float32),
    }
    print("result:", kernel(**inputs))


# revision 4
# speedup vs baseline: 1.2793x; 1.2793x over previous
"""Trainium2 Bass kernel for nn_MEPG_Loss (MEPG policy-gradient loss).

Math (forward only; stop_gradient is identity):
    h   = tanh(states[s,:,t] @ W1 + b1)                  [S,T,H]
    mu  = h @ W2 + b2                                    [S,T,A]
    ll[s,t] = -0.5*(||a[s,:,t]-mu||^2/SD + A*log(2*pi*SD))
    base = rewards.T - ALPHA*ll.T ; cum = base with row T-2 += row T-1
    A_hat = cum - log(0.5)
    out = einsum('ts,us->', A_hat, ll.T)/S
        = sum_s (sum_t A_hat[t,s]) * (sum_t ll[t,s]) / S

Only per-simulation reductions are needed:
    q_sum[s]  = sum_{t,d} (mu - a)^2,   q_last[s] = sum_d (mu - a)^2 at t=T-1
    R[s] = sum_t rewards,               r_last[s] = rewards[s,T-1]
(R/r_last come straight from host numpy; rewards are never sent to the device.)

Device pipeline, per core (256 sims as 64 quads of 4 sims):
    - states prepacked on host to [64, NQ*T] bf16; per 4-quad block one
      contiguous DMA per sim-slot j lands at SBUF partitions {32j..32j+16}
    - mm1: 4 row-tiled K=16 matmuls (concurrent via tile_position) fill a
      4-bank PSUM tile hp [128, 4*T] with h_pre for the whole quad
    - ScalarE: ONE tanh activation over all 2048 columns (bias=b1) -> h bf16
    - mm2: 4 col-tiled matmuls (lhsT=W2, start) write mu into hp[:, 0:T],
      reusing the first bank of the already-consumed h_pre tile
    - diff: 4 diag-tiled identity matmuls accumulate (b2 - a) onto mu
    - DVE: bn_stats + bn_aggr give per-partition mean/var of diff over T
      (=> sum of squares), plus a 1-col copy of diff at t=T-1
Final combine (tiny) is done on host in float64.

ScalarE is the bottleneck engine (tanh, 1 elem/lane/cycle): ~118 us floor.
Everything else is sized to stay below that and overlap fully.
"""

import os
import sys

import numpy as np

if not any(os.path.isdir(os.path.join(p, "concourse")) for p in sys.path if p):
    sys.path.insert(0, "/opt/trn_rl_repo")

import ml_dtypes

import concourse.bacc as bacc
import concourse.tile as tile
from concourse import mybir
from concourse.bass_utils import run_bass_kernel_spmd

# Problem constants (hardcoded per contract)
S, D, A, T, HID = 2048, 16, 4, 512, 128
N_CORES = 8
SS = S // N_CORES          # 256 sims per core
NQ = SS // 4               # 64 quads per core
QB = 4                     # quads per DMA block
NB = NQ // QB              # 16 blocks
SD_VAR = 0.04
ALPHA = 0.1
MAX_POSITION = 1.0

F32 = mybir.dt.float32
BF16 = mybir.dt.bfloat16
NP_BF16 = ml_dtypes.bfloat16


def _build_program():
    nc = bacc.Bacc("TRN2", target_bir_lowering=False, debug=False)

    stp_d = nc.dram_tensor("st_pre", [64, NQ * T], BF16, kind="ExternalInput").ap()
    atp_d = nc.dram_tensor("at_pre", [16, NQ * T], BF16, kind="ExternalInput").ap()
    w1f_d = nc.dram_tensor("w1full", [128, HID], BF16, kind="ExternalInput").ap()
    w2_d = nc.dram_tensor("w2", [HID, A], BF16, kind="ExternalInput").ap()
    id4_d = nc.dram_tensor("id4", [128, A], BF16, kind="ExternalInput").ap()
    b1_d = nc.dram_tensor("b1col", [HID, 1], F32, kind="ExternalInput").ap()

    mv_d = nc.dram_tensor("mv", [128, 2 * NQ], F32, kind="ExternalOutput").ap()
    ql_d = nc.dram_tensor("ql", [128, NQ], F32, kind="ExternalOutput").ap()

    with tile.TileContext(nc) as tc:
        with (
            tc.tile_pool(name="consts", bufs=1) as consts,
            tc.tile_pool(name="stp", bufs=3) as stp,
            tc.tile_pool(name="atp", bufs=3) as atp,
            tc.tile_pool(name="hsb", bufs=2) as hsb,
            tc.tile_pool(name="bstp", bufs=2) as bstp,
            tc.tile_pool(name="outs", bufs=1) as outp,
            tc.tile_pool(name="hpp", bufs=1, space="PSUM") as hpp,
        ):
            # dummy activation: forces the tanh table load at t~0, off the
            # critical path (no data dependence)
            dums = consts.tile([128, 1], F32, tag="dums")
            dumo = consts.tile([128, 1], F32, tag="dumo")
            nc.vector.memset(dums[:], 0.0)
            nc.scalar.activation(
                out=dumo[:], in_=dums[:],
                func=mybir.ActivationFunctionType.Tanh, scale=1.0,
            )

            # constants
            w1t = consts.tile([128, HID], BF16, tag="w1t")
            w2t = consts.tile([HID, A], BF16, tag="w2t")
            id4t = consts.tile([128, A], BF16, tag="id4t")
            b1t = consts.tile([HID, 1], F32, tag="b1t")
            nc.sync.dma_start(out=w1t[:], in_=w1f_d)
            nc.sync.dma_start(out=w2t[:], in_=w2_d)
            nc.sync.dma_start(out=id4t[:], in_=id4_d)
            nc.sync.dma_start(out=b1t[:], in_=b1_d)

            mv_sb = outp.tile([128, 2 * NQ], F32, tag="mv")
            ql_sb = outp.tile([128, NQ], F32, tag="ql")

            def _tail_quad(g, h, hp, at):
                q = g % QB
                mu = hp[:, 0:T]
                for j in range(4):
                    nc.tensor.matmul(
                        out=mu[32 * j:32 * j + A, :],
                        lhsT=w2t[:],
                        rhs=h[:, T * j:T * (j + 1)],
                        start=True, stop=False,
                        tile_position=(0, 32 * j),
                        skip_group_check=True,
                    )
                for j in range(4):
                    nc.tensor.matmul(
                        out=mu[32 * j:32 * j + A, :],
                        lhsT=id4t[32 * j:32 * j + A, :],
                        rhs=at[32 * j:32 * j + A, T * q:T * (q + 1)],
                        start=False, stop=True,
                        tile_position=(32 * j, 32 * j),
                        skip_group_check=True,
                    )
                # bn_stats + qlast copy are the last readers of hp(g); keep
                # them early and keep bn_aggr (SBUF->SBUF) off that path
                sts = bstp.tile([128, 6], F32, tag="bst", name=f"bst_{g}")
                nc.vector.tensor_copy(ql_sb[:, g:g + 1], mu[:, T - 1:T])
                nc.vector.bn_stats(out=sts[:], in_=mu[:])
                nc.vector.bn_aggr(out=mv_sb[:, 2 * g:2 * g + 2], in_=sts[:])

            pipe = None
            for b in range(NB):
                c0 = QB * T * b
                st = stp.tile([128, QB * T], BF16, tag="st", name=f"st_{b}")
                at = atp.tile([128, QB * T], BF16, tag="at", name=f"at_{b}")
                for j in range(4):
                    nc.sync.dma_start(
                        out=st[32 * j:32 * j + D, :],
                        in_=stp_d[D * j:D * (j + 1), c0:c0 + QB * T],
                    )
                    nc.gpsimd.dma_start(
                        out=at[32 * j:32 * j + A, :],
                        in_=atp_d[A * j:A * (j + 1), c0:c0 + QB * T],
                    )
                for q in range(QB):
                    g = QB * b + q
                    # tail(g-1) FIRST: on the strict-FIFO PE queue, mm2/diff
                    # of the previous quad must sit ahead of mm1(g), else
                    # mm1(g)'s wait (psum-buffer reuse, gated on bn_stats of
                    # g-2) head-of-line-blocks them and serializes the loop
                    if pipe is not None:
                        _tail_quad(*pipe)
                    hp = hpp.tile([128, 4 * T], F32, tag=f"hp{g % 2}",
                                  name=f"hp_{g}")
                    for j in range(4):
                        nc.tensor.matmul(
                            out=hp[:, T * j:T * (j + 1)],
                            lhsT=w1t[32 * j:32 * j + D, :],
                            rhs=st[32 * j:32 * j + D, T * q:T * (q + 1)],
                            start=True, stop=True,
                            tile_position=(32 * j, 0),
                        )
                    h = hsb.tile([128, 4 * T], BF16, tag="h", name=f"h_{g}")
                    nc.scalar.activation(
                        out=h[:], in_=hp[:],
                        func=mybir.ActivationFunctionType.Tanh,
                        bias=b1t[:], scale=1.0,
                    )
                    pipe = (g, h, hp, at)

            if pipe is not None:
                _tail_quad(*pipe)

            nc.sync.dma_start(out=mv_d, in_=mv_sb[:])
            nc.sync.dma_start(out=ql_d, in_=ql_sb[:])

    nc.finalize()
    return nc


_NC_CACHE = {}


def _get_program():
    if "nc" not in _NC_CACHE:
        _NC_CACHE["nc"] = _build_program()
    return _NC_CACHE["nc"]


def _make_consts(W1, b1, W2):
    w1full = np.zeros((128, HID), dtype=NP_BF16)
    id4 = np.zeros((128, A), dtype=NP_BF16)
    for j in range(4):
        w1full[32 * j:32 * j + D, :] = W1.astype(NP_BF16)
        for d in range(A):
            id4[32 * j + d, d] = 1.0
    return {
        "w1full": w1full,
        "w2": np.ascontiguousarray(W2.astype(NP_BF16)),
        "id4": id4,
        "b1col": np.ascontiguousarray(b1.astype(np.float32).reshape(HID, 1)),
    }


def kernel(states, actions, rewards, W1, b1, W2, b2, _run_kwargs=None):
    states = np.asarray(states, dtype=np.float32)
    actions = np.asarray(actions, dtype=np.float32)
    rewards = np.asarray(rewards, dtype=np.float32)
    W1 = np.asarray(W1, dtype=np.float32)
    b1 = np.asarray(b1, dtype=np.float32)
    W2 = np.asarray(W2, dtype=np.float32)
    b2 = np.asarray(b2, dtype=np.float32)

    consts = _make_consts(W1, b1, W2)

    # prepack per-core device layouts:
    #   st_pre[16j+dd, g*T+t] = states[core*SS + 4g+j, dd, t]   (bf16)
    #   at_pre[4j+d,  g*T+t] = b2[d] - actions[core*SS + 4g+j, d, t]  (bf16)
    st_all = states.reshape(N_CORES, SS // 4, 4, D, T)
    st_all = np.ascontiguousarray(st_all.transpose(0, 2, 3, 1, 4)).astype(NP_BF16)
    st_all = st_all.reshape(N_CORES, 64, NQ * T)
    aadj = b2[None, :, None] - actions
    at_all = aadj.reshape(N_CORES, SS // 4, 4, A, T)
    at_all = np.ascontiguousarray(at_all.transpose(0, 2, 3, 1, 4)).astype(NP_BF16)
    at_all = at_all.reshape(N_CORES, 16, NQ * T)

    in_maps = []
    for c in range(N_CORES):
        m = {"st_pre": st_all[c], "at_pre": at_all[c]}
        m.update(consts)
        in_maps.append(m)

    nc = _get_program()
    res = run_bass_kernel_spmd(nc, in_maps, core_ids=list(range(N_CORES)),
                               **(_run_kwargs or {}))
    results = res.results

    # host combine in float64
    C0 = -0.5 * A * np.log(2.0 * np.pi * SD_VAR)
    mx_pos = np.log(1.0 / (2.0 * MAX_POSITION))
    rew = rewards.astype(np.float64)
    R_all = rew.sum(axis=1)            # [S]
    rlast_all = rew[:, -1]             # [S]
    total = 0.0
    for c in range(N_CORES):
        mv = results[c]["mv"].astype(np.float64)      # [128, 2*NQ]
        qlv = results[c]["ql"].astype(np.float64)     # [128, NQ]
        mean = mv[:, 0::2]                            # [128, NQ]
        var = mv[:, 1::2]
        sumsq = T * (var + mean * mean)               # Sum_t diff^2 per (p, g)
        # partition p = 32j + d (d < A), sim s_local = 4g + j
        sel = sumsq.reshape(4, 32, NQ)[:, :A, :]      # [j, d, g]
        q_sum = sel.sum(axis=1).T.reshape(SS)         # s_local = 4g + j
        qsel = qlv.reshape(4, 32, NQ)[:, :A, :]
        q_last = (qsel ** 2).sum(axis=1).T.reshape(SS)
        sl = slice(SS * c, SS * (c + 1))
        L = -0.5 * q_sum / SD_VAR + T * C0
        ll_last = -0.5 * q_last / SD_VAR + C0
        A_sum = (R_all[sl] + rlast_all[sl]
                 - ALPHA * (L + ll_last) - T * mx_pos)
        total += np.sum(A_sum * L)
    out = np.float32(total / S)
    if _run_kwargs:
        _NC_CACHE["last_result"] = res
    return out


if __name__ == "__main__":
    rng = np.random.default_rng(0)
    inputs = {
        "states": rng.standard_normal((S, D, T), dtype=np.float32),
        "actions": rng.standard_normal((S, A, T), dtype=np.float32),
        "rewards": rng.standard_normal((S, T), dtype=np.float32),
        "W1": (rng.standard_normal((D, HID)) / np.sqrt(D)).astype(np.float32),
        "b1": np.zeros(HID, np.float32),
        "W2": (rng.standard_normal((HID, A)) / np.sqrt(HID)).astype(np.float32),
        "b2": np.zeros(A, np.float32),
    }
    print("result:", kernel(**inputs))
